# revision 18
# baseline (speedup 1.0000x reference)
"""Trainium2 Bass kernel for the EnforcedNeuralODE recurrence.

Reference (per timestep): x_t = Wx x_{t-1} + Wf f_{t-1} + b over T-1=4095
steps, batch 256, state 64, force 64.  Output [T, B, 64].

Algorithm (per core, 32-sample batch shard, all math bf16 / f32 PSUM):
  Bias fold: f'_t = f_t + Wf^{-1} b, so x_t = Wx x_{t-1} + Wf f'_t.
  Blocks of KB=32 steps; NB=128 blocks; chunk = 16 blocks (free dim
  N=512 cols = 16 blocks x 32 batch); 8 chunks in pipeline groups
  GROUPS=[2,2,4] (small first group shortens the DMA lead-in, wide
  last group gives the x-chain enough PE spacing without a partner).
  Phase1  g31_blk = sum_j Wx^{31-j} Wf f'_j   (block forcing response)
  P2      block start states s_b: superblock (8 blocks) convolution +
          scan + 7-step parallel reconstruction of interior entries.
  Phase2  x-chain per block pair-by-pair, two matmuls per pair tile
          [x_odd; x_even] (M=128), chained through bf16 out staging:
            x_{2p+1} = Wx^2 x_{2p-1} + WxWf f'_{2p} + Wf f'_{2p+1}
            x_{2p}   = Wx   x_{2p-1} + Wf   f'_{2p}
  All matmuls K=128/M=128 (zero-padded lhsT), bf16 operands: uniform
  tile mode; LDWEIGHTS overlaps the previous matmul.
  Group pipeline: f DMA (graded pieces over sync/scalar/gpsimd/vector
  queues) -> phase1 -> P2 -> phase2 (next group's phase1 interleaved
  into the sweeps to keep PE dense) -> per-sweep out DMA.
"""

import numpy as np
from contextlib import ExitStack

NCORES = 8
BATCH, STATE, FDIM, TIMESPAN = 256, 64, 64, 4096

BC = BATCH // NCORES    # 32 batch per core
KB = 32                 # steps per block
PAIRS = KB // 2         # 16 step-pairs per block
NB = TIMESPAN // KB     # 128 blocks
NBC = 16                # blocks per chunk
CHUNKS = NB // NBC      # 8
N = NBC * BC            # 512 free cols per (chunk, pair)
GROUPS = [4, 4]         # chunks per pipeline group
GOFF = [0, 4]           # first chunk of each group
SBK = 8                 # blocks per superblock (P2)

F_COLS = PAIRS * CHUNKS * N       # 65536 forcing cols (bf16)
O_COLS = PAIRS * CHUNKS * N       # 65536 output cols (bf16)
W_COLS = (PAIRS + 2 + 8 + 1) * 128  # 3456 weight cols
# f DMA piece sizes (in pairs) per group; alternating sync/scalar
F_PIECES = {4: [4, 4, 4, 4]}

_NC_CACHE: dict = {}


def _gbase(G):
    """first column block index (pair-chunk units) of group G"""
    return sum(PAIRS * w for w in GROUPS[:G])


def _build_nc():
    import concourse.bass as bass  # noqa: F401
    import concourse.tile as tile
    from concourse import bacc, mybir

    f32 = mybir.dt.float32
    bf16 = mybir.dt.bfloat16
    AF = mybir.ActivationFunctionType

    nc = bacc.Bacc("TRN2", target_bir_lowering=False, debug=False)

    f_dram = nc.dram_tensor("f", [128, F_COLS], bf16, kind="ExternalInput")
    w_dram = nc.dram_tensor("wts", [128, W_COLS], bf16, kind="ExternalInput")
    s0_dram = nc.dram_tensor("s0", [128, BC], bf16, kind="ExternalInput")
    out_dram = nc.dram_tensor("out", [128, O_COLS], bf16, kind="ExternalOutput")

    with tile.TileContext(nc) as tc, ExitStack() as ctx:
        singles = ctx.enter_context(tc.tile_pool(name="singles", bufs=1))
        opool = ctx.enter_context(tc.tile_pool(name="opool", bufs=4))
        psA = ctx.enter_context(tc.tile_pool(name="psA", bufs=4, space="PSUM"))
        psB = ctx.enter_context(tc.tile_pool(name="psB", bufs=4, space="PSUM"))

        fsb = singles.tile([128, F_COLS], bf16)
        wsb = singles.tile([128, W_COLS], bf16)
        s_sb = singles.tile([128, (NB + 1) * BC], bf16)
        g31 = singles.tile([128, NB * BC], bf16)
        vsb = singles.tile([128, (NB // SBK) * BC], bf16)

        def L1(p):
            return wsb[:, p * 128 : (p + 1) * 128]

        Lhx = wsb[:, 2048:2176]
        Lf = wsb[:, 2176:2304]

        def Lj(j):
            return wsb[:, 2304 + j * 128 : 2304 + (j + 1) * 128]

        Lrec = Lj(6)          # (Wx^32)^T
        Lscan = wsb[:, 3328:3456]

        def fv(G, p, ci):
            base = (_gbase(G) + p * GROUPS[G] + ci) * N
            return fsb[:, base : base + N]

        # ---- input DMAs: graded f pieces round-robin over 4 queues ----
        nc.scalar.dma_start(out=wsb[:], in_=w_dram[:])
        nc.scalar.dma_start(out=s_sb[:, 0:BC], in_=s0_dram[:])
        for G, W in enumerate(GROUPS):
            # partition-split f pieces: the even/odd engine octets serve
            # partition halves, so the two queues stream concurrently on
            # disjoint engines with descriptors of seg_cols*2 bytes
            for pa, pb in [(0, 4), (4, 10), (10, 16)]:
                c0 = (_gbase(G) + pa * W) * N
                c1 = (_gbase(G) + pb * W) * N
                nc.sync.dma_start(out=fsb[0:64, c0:c1], in_=f_dram[0:64, c0:c1])
                nc.scalar.dma_start(
                    out=fsb[64:128, c0:c1], in_=f_dram[64:128, c0:c1]
                )

        def g31v(G, j, sub=None):
            """blocks {B0 + 8s + j} for superblocks s of group G: [128, SBH, 32]"""
            B0 = GOFF[G] * NBC
            nblk = GROUPS[G] * NBC
            r = g31[:, B0 * BC : (B0 + nblk) * BC].rearrange(
                "p (s j b) -> p s j b", s=nblk // SBK, j=SBK, b=BC
            )
            return r[:, :, j, :]

        def sv(G, k):
            B0 = GOFF[G] * NBC
            nblk = GROUPS[G] * NBC
            r = s_sb[:, B0 * BC : (B0 + nblk) * BC].rearrange(
                "p (s j b) -> p s j b", s=nblk // SBK, j=SBK, b=BC
            )
            return r[:, :, k, :]

        def phase1_evac(G, ci, acc, eng):
            c = GOFF[G] + ci
            if eng == 0:
                nc.scalar.activation(g31[:, c * N : (c + 1) * N], acc[:], AF.Copy)
            else:
                nc.vector.tensor_copy(g31[:, c * N : (c + 1) * N], acc[:])

        # ---- phase1 group 0 (sweep-major: follows f DMA arrival) ----
        W0 = GROUPS[0]
        accs0 = [psA.tile([128, N], f32, tag="A", name=f"acc0_{ci}") for ci in range(W0)]
        for p in range(PAIRS):
            for ci in range(W0):
                nc.tensor.matmul(
                    accs0[ci][:], L1(p), fv(0, p, ci),
                    start=(p == 0), stop=(p == PAIRS - 1),
                )
        for ci in range(W0):
            phase1_evac(0, ci, accs0[ci], ci % 2)

        # ---- P2 for one group ----
        def p2(G):
            B0 = GOFF[G] * NBC
            SBH = GROUPS[G] * NBC // SBK
            vt = psA.tile([128, N], f32, tag="A", name="vt")
            for j in range(SBK):
                nc.tensor.matmul(
                    vt[:, 0 : SBH * BC], Lj(j), g31v(G, j),
                    start=(j == 0), stop=(j == SBK - 1),
                )
            S0 = B0 // SBK
            nc.scalar.activation(
                vsb[:, S0 * BC : (S0 + SBH) * BC], vt[:, 0 : SBH * BC], AF.Copy
            )
            for s in range(SBH):
                S = S0 + s
                e_in, e_out = S * SBK, (S + 1) * SBK
                pt = psA.tile([128, N], f32, tag="A", name="pt")
                nc.tensor.matmul(
                    pt[:, 0:BC], Lscan,
                    s_sb[:, e_in * BC : (e_in + 1) * BC],
                    start=True, stop=True,
                )
                nc.vector.tensor_add(
                    s_sb[:, e_out * BC : (e_out + 1) * BC],
                    pt[:, 0:BC],
                    vsb[:, S * BC : (S + 1) * BC],
                )
            for k in range(1, SBK):
                rt = psA.tile([128, N], f32, tag="A", name="rt")
                nc.tensor.matmul(
                    rt[:, 0 : SBH * BC], Lrec, sv(G, k - 1), start=True, stop=True
                )
                nc.vector.tensor_add(sv(G, k), rt[:, 0 : SBH * BC], g31v(G, k - 1))

        p2(0)

        # ---- phase2 for group G; interleave next group's phase1 ----
        def phase2(G):
            W = GROUPS[G]
            nxt = G + 1 if G + 1 < len(GROUPS) else None
            Wn = GROUPS[nxt] if nxt is not None else 0
            il_accs = {}
            ost = None
            prev_slices = None
            for p in range(PAIRS):
                pe = p % 2
                if pe == 0:
                    ost = opool.tile([128, 2 * W * N], bf16, tag="ost", name="ost")
                chain = [
                    psB.tile([128, N], f32, tag="B", name=f"ch{ci}") for ci in range(W)
                ]
                for ci in range(W):
                    if p == 0:
                        c = GOFF[G] + ci
                        prev = s_sb[:, c * NBC * BC : (c + 1) * NBC * BC]
                    else:
                        prev = prev_slices[ci]
                    nc.tensor.matmul(chain[ci][:], Lhx, prev, start=True, stop=False)
                for ci in range(W):
                    nc.tensor.matmul(
                        chain[ci][:], Lf, fv(G, p, ci), start=False, stop=True
                    )
                if nxt is not None:
                    # next group's phase1: halves of its chunks get 8
                    # sweeps each, 2 pairs per chunk per sweep
                    half = Wn // 2
                    cpair = p // 8
                    q0 = (p % 8) * 2
                    for dci in range(half):
                        ci1 = cpair * half + dci
                        if q0 == 0:
                            il_accs[ci1] = psA.tile(
                                [128, N], f32, tag="A", name=f"il{ci1}"
                            )
                        for q in (q0, q0 + 1):
                            nc.tensor.matmul(
                                il_accs[ci1][:], L1(q), fv(nxt, q, ci1),
                                start=(q == 0), stop=(q == PAIRS - 1),
                            )
                    if p % 8 == 7:
                        for dci in range(half):
                            ci1 = cpair * half + dci
                            phase1_evac(nxt, ci1, il_accs[ci1], dci % 2)
                prev_slices = []
                for ci in range(W):
                    dst = ost[:, (pe * W + ci) * N : (pe * W + ci + 1) * N]
                    prev_slices.append(dst)
                    if (p * W + ci) % 2 == 0:
                        nc.scalar.activation(dst, chain[ci][:], AF.Copy)
                    else:
                        nc.vector.tensor_copy(dst, chain[ci][:])
                if pe == 1:
                    base = (_gbase(G) + (p - 1) * W) * N
                    cols = 2 * W * N
                    last = G == len(GROUPS) - 1 and p == PAIRS - 1
                    if last:
                        # split the final store across two queues
                        nc.gpsimd.dma_start(
                            out=out_dram[:, base : base + cols // 2],
                            in_=ost[:, 0 : cols // 2],
                        )
                        nc.sync.dma_start(
                            out=out_dram[:, base + cols // 2 : base + cols],
                            in_=ost[:, cols // 2 :],
                        )
                    else:
                        oq = [nc.gpsimd, nc.sync]
                        oq[(_gbase(G) // PAIRS + p // 2) % 2].dma_start(
                            out=out_dram[:, base : base + cols], in_=ost[:]
                        )

        for G in range(len(GROUPS)):
            if G > 0:
                p2(G)
            phase2(G)

    nc.compile()
    return nc


def _get_nc():
    if "nc" not in _NC_CACHE:
        _NC_CACHE["nc"] = _build_nc()
    return _NC_CACHE["nc"]


def _host_prep(inputs, forcing, fc_w, fc_b):
    """Build per-core input maps (numpy only, untimed)."""
    import ml_dtypes

    bf = ml_dtypes.bfloat16
    inputs = np.asarray(inputs, np.float32)
    fc_w = np.asarray(fc_w, np.float32)
    fc_b = np.asarray(fc_b, np.float32)
    Wx = fc_w[:, :STATE].astype(np.float64)
    Wf = fc_w[:, STATE:].astype(np.float64)
    b = fc_b.astype(np.float64)
    c = np.linalg.solve(Wf, b)

    WxP = {}
    P = np.eye(STATE)
    for j in range(33):
        WxP[j] = P
        P = Wx @ P
    W256 = np.linalg.matrix_power(Wx, 256)

    wts = np.zeros((128, W_COLS), np.float32)
    for p in range(PAIRS):
        wts[0:64, p * 128 : p * 128 + 64] = (WxP[31 - 2 * p] @ Wf).T
        wts[64:128, p * 128 : p * 128 + 64] = (WxP[30 - 2 * p] @ Wf).T
    wts[0:64, 2048:2112] = (WxP[2]).T          # Lhx: x_odd <- Wx^2 x
    wts[0:64, 2112:2176] = Wx.T                # Lhx: x_even <- Wx x
    wts[0:64, 2176:2240] = (Wx @ Wf).T         # Lf: x_odd <- WxWf f0
    wts[0:64, 2240:2304] = Wf.T                # Lf: x_even <- Wf f0
    wts[64:128, 2176:2240] = Wf.T              # Lf: x_odd <- Wf f1
    for j in range(8):
        wts[0:64, 2304 + j * 128 : 2304 + j * 128 + 64] = (
            np.linalg.matrix_power(Wx, (7 - j) * KB)
        ).T
    wts[0:64, 3328:3392] = W256.T
    wts = wts.astype(bf)

    fp = np.zeros((TIMESPAN, BATCH, FDIM), np.float32)
    fp[: TIMESPAN - 1] = np.asarray(forcing, np.float32) + c.astype(np.float32)
    fp[TIMESPAN - 1] = c.astype(np.float32)
    # [Bk, pair, parity, batch, feat]; Bk = (GOFF[G]+ci)*16 + blk
    arr = fp.reshape(NB, PAIRS, 2, BATCH, FDIM)

    in_maps = []
    for core in range(NCORES):
        bs = slice(core * BC, (core + 1) * BC)
        fcore = np.empty((128, F_COLS), bf)
        for G, W in enumerate(GROUPS):
            for p in range(PAIRS):
                for ci in range(W):
                    c0 = (_gbase(G) + p * W + ci) * N
                    Bk0 = (GOFF[G] + ci) * NBC
                    # [blk, par, b, feat] -> [par*64+feat, blk*32+b]
                    blkarr = arr[Bk0 : Bk0 + NBC, p, :, bs, :]
                    blkarr = blkarr.transpose(1, 3, 0, 2).reshape(128, N)
                    fcore[:, c0 : c0 + N] = blkarr.astype(bf)
        s0 = np.zeros((128, BC), np.float32)
        s0[0:64] = inputs[bs].T
        in_maps.append({"f": fcore, "wts": wts, "s0": s0.astype(bf)})
    return in_maps


def _host_decode(results, inputs):
    """Per-core out [128, O_COLS] bf16 -> full [T, B, S] f32."""
    inputs = np.asarray(inputs, np.float32)
    out = np.empty((TIMESPAN, BATCH, STATE), np.float32)
    out[0] = inputs
    for core in range(NCORES):
        o = np.asarray(results[core]["out"], dtype=np.float32)
        o = o.reshape(2, 64, O_COLS)  # [par, feat, col]
        ocore = np.empty((TIMESPAN, BC, STATE), np.float32)
        for G, W in enumerate(GROUPS):
            for p in range(PAIRS):
                for ci in range(W):
                    c0 = (_gbase(G) + p * W + ci) * N
                    blk = o[:, :, c0 : c0 + N].reshape(2, 64, NBC, BC)
                    Bk0 = (GOFF[G] + ci) * NBC
                    ts = (np.arange(NBC) + Bk0) * KB + 2 * p
                    # par 1 = x_{2p} (t+0), par 0 = x_{2p+1} (t+1)
                    ocore[ts, :, :] = blk[1].transpose(1, 2, 0)
                    ocore[ts + 1, :, :] = blk[0].transpose(1, 2, 0)
        out[1:, core * BC : (core + 1) * BC] = ocore[: TIMESPAN - 1]
    return out


def kernel(inputs, forcing, fc_w, fc_b, timespan):
    from concourse.bass_utils import run_bass_kernel_spmd

    timespan = int(timespan)
    assert timespan == TIMESPAN, f"hardcoded for timespan={TIMESPAN}, got {timespan}"
    nc = _get_nc()
    in_maps = _host_prep(inputs, forcing, fc_w, fc_b)
    res = run_bass_kernel_spmd(nc, in_maps, core_ids=list(range(NCORES)))
    return _host_decode(res.results, inputs)


if __name__ == "__main__":
    nc = _get_nc()
    print("built ok")
